# revision 22
# baseline (speedup 1.0000x reference)
"""Trainium2 Bass kernel for MiniMax softmax attention (T=4096, H=4096, 32 q heads,
8 kv heads, head_dim=128, partial neox RoPE, causal softmax, o_proj).

Sharding: tensor-parallel over heads across 8 NeuronCores. Core c computes q heads
4c..4c+3 (= kv-head group c). Host sums the 8 partial outputs (row-parallel o_proj).

v2 design (vs f32r baseline):
  * all matmuls in bf16 (fp32 PSUM accumulation). f32r matmuls self-load weights
    serially (~287ns/MM measured vs 213ns stream floor); bf16 pipelines LDWEIGHTS
    under the previous MM stream. Accuracy headroom is large (gate 2e-2).
  * softmax denominator: no 128x1xN PE matmuls per key-tile (148us of PE in the
    baseline). Instead exp tiles are accumulated lane-wise on DVE in bf16 via
    scalar_tensor_tensor (4x_2p mode), then ONE ones[128,128] matmul per
    (head, q-chunk) partition-reduces the accumulator and broadcasts the
    denominator to 128 partitions (feeds reciprocal+normalize directly).
  * o_proj fully fused on-chip: normalized attention stays in SBUF (bf16),
    no DRAM spill round-trip; output partials written bf16 (halves writeback).
  * v transposed via DMA xbar transpose (off-engine) instead of PE transposes.
  * attention runs in 2-head passes (av 2 + ss 2x2 = 6 PSUM banks) leaving 2
    banks for the next chunk's qkv projection to overlap ACT-bound softmax.
  * hidden_t is cached per t-chunk in SBUF (bf16) so the qkv projection can run
    in 3 j-groups of 2 PSUM banks without re-reading HBM.
"""
import numpy as np

T = 4096
HIDDEN = 4096
NH = 32
NKV = 8
HD = 128
RD = 64
HALF = 32
ROPE_BASE = 10000000.0
NC_CORES = 8
HPC = NH // NC_CORES      # 4 q heads per core
QC = 512                  # query chunk
NTC = T // QC             # 8 t-chunks
NKO = 32                  # hidden contraction chunks of 128
NJ = HPC + 2              # 6 j-tiles of 128 per core

_CACHE = {}


def _build_nc():
    import concourse.mybir as mybir
    import concourse.tile as tile
    from concourse import bacc

    F32 = mybir.dt.float32
    BF16 = mybir.dt.bfloat16
    EXP = mybir.ActivationFunctionType.Exp
    LOG = mybir.ActivationFunctionType.Ln
    MUL = mybir.AluOpType.mult
    ADD = mybir.AluOpType.add

    from concourse.bass import broadcast_tensor_aps

    nc = bacc.Bacc()
    hidden_t = nc.dram_tensor("hidden_t", [HIDDEN, T], BF16, kind="ExternalInput")
    w_qkvp = nc.dram_tensor("w_qkvp", [HIDDEN, NJ * HD], BF16, kind="ExternalInput")
    w_op = nc.dram_tensor("w_op", [HPC * HD, HIDDEN], BF16, kind="ExternalInput")
    cs_a = nc.dram_tensor("cs_a", [RD, T], BF16, kind="ExternalInput")
    cs_b = nc.dram_tensor("cs_b", [RD, T], BF16, kind="ExternalInput")
    dmask2 = nc.dram_tensor("dmask2", [128, 2 * 128], BF16, kind="ExternalInput")
    out_p = nc.dram_tensor("out_p", [T, HIDDEN], BF16, kind="ExternalOutput")

    with tile.TileContext(nc) as tc:
        with (
            tc.tile_pool(name="const", bufs=1) as const,
            tc.tile_pool(name="kv", bufs=1) as kvp,
            tc.tile_pool(name="ht", bufs=1) as htp,
            tc.tile_pool(name="qt", bufs=1) as qtp,
            tc.tile_pool(name="rope", bufs=1) as ropep,
            tc.tile_pool(name="ex", bufs=1) as exp_pool,
            tc.tile_pool(name="misc", bufs=1) as miscp,
            tc.tile_pool(name="attn", bufs=1) as attnp,
            tc.tile_pool(name="psa", bufs=1, space="PSUM") as psa,
        ):
            # ---- constants (DMAs for cs/dmask deferred below the first ht/w
            # loads so the first qkv matmul isn't queued behind them)
            csa_sb = const.tile([RD, T], BF16, name="csa", tag="csa")
            csb_sb = const.tile([RD, T], BF16, name="csb", tag="csb")
            dmask_sb = const.tile([128, 2, 128], BF16, name="dmask", tag="dmask")
            ones_sb = const.tile([128, 128], BF16, name="ones", tag="ones")
            nc.gpsimd.memset(ones_sb[:], 1.0)

            # kv cache: one tile per t-chunk to keep dependency tracking fine-grained
            kT_tiles = []     # [128 d, 512 keys] bf16 (roped)
            v_tiles = []      # [128 keys(sub), 4 sub, 128 d] bf16
            attn_tiles = []   # [128 d, 4 h, 512 t] bf16 normalized attention^T
            for i in range(NTC):
                kT_tiles.append(kvp.tile([128, QC], BF16, name=f"kT{i}", tag=f"kT{i}"))
                v_tiles.append(kvp.tile([128, 4, HD], BF16, name=f"v{i}", tag=f"v{i}"))
                attn_tiles.append(
                    attnp.tile([128, HPC, QC], BF16, name=f"at{i}", tag=f"at{i}")
                )

            with tc.tile_pool(name="w", bufs=1) as wp, \
                 tc.tile_pool(name="psq", bufs=1, space="PSUM") as psq:
                w_sb = wp.tile([128, NKO, NJ * HD], BF16, name="w")
                w_view = w_qkvp[:].rearrange("(ko p) j -> p ko j", p=128)

                ht_view = hidden_t[:].rearrange("(ko p) t -> p ko t", p=128)
                ht_tiles = {}

                def load_ht_q(tci, q):
                    if tci >= NTC or (tci, q) in ht_tiles:
                        return
                    tsl = slice(tci * QC, (tci + 1) * QC)
                    htt = htp.tile([128, 8, QC], BF16, name="ht", tag="ht", bufs=6)
                    nc.sync.dma_start(htt[:], ht_view[:, q * 8:(q + 1) * 8, tsl])
                    ht_tiles[(tci, q)] = htt

                # first compute deps first: ht(0) quarter 0 + w chunk 0
                nc.sync.dma_start(w_sb[:, 0:4, :], w_view[:, 0:4, :])
                for q in range(4):
                    load_ht_q(0, q)
                for wi in range(1, 8):
                    nc.sync.dma_start(
                        w_sb[:, wi * 4:(wi + 1) * 4, :], w_view[:, wi * 4:(wi + 1) * 4, :]
                    )
                nc.sync.dma_start(csa_sb[:], cs_a[:])
                nc.sync.dma_start(csb_sb[:], cs_b[:])
                nc.sync.dma_start(
                    dmask_sb[:], dmask2[:].rearrange("p (two t) -> p two t", two=2)
                )
                load_ht_q(1, 0)
                load_ht_q(1, 1)
                qcur_tiles = {}

                def do_qkv(tci):
                    """qkv^T projection for t-chunk tci + rope + kv-cache fill."""
                    tsl = slice(tci * QC, (tci + 1) * QC)
                    qcur = qtp.tile([128, HPC, QC], BF16, name="qcur", tag="qt", bufs=2)
                    qcur_tiles[tci] = qcur

                    def rope_pair(dst):
                        # in-place neox rope on dst[0:64, pair, t]; dst bf16
                        # dst: [128, 2, 512]; tables broadcast over the pair dim
                        x1 = dst[:HALF]
                        x2 = dst[HALF:RD]
                        sh = [HALF, 2, QC]
                        t1 = ropep.tile(sh, BF16, name="r1", tag="r1", bufs=1)
                        t2 = ropep.tile(sh, BF16, name="r2", tag="r2", bufs=1)
                        t3 = ropep.tile(sh, BF16, name="r3", tag="r3", bufs=1)
                        t4 = ropep.tile(sh, BF16, name="r4", tag="r4", bufs=1)

                        def bc(tab):  # [32, 512] -> [32, 2(0-stride), 512]
                            t3d = tab.rearrange("p (one t) -> p one t", one=1)
                            b, _ = broadcast_tensor_aps(t3d, x1)
                            return b

                        cosb = bc(csa_sb[:HALF, tsl])
                        sinb2 = bc(csa_sb[HALF:, tsl])
                        sinb = bc(csb_sb[:HALF, tsl])
                        cosb2 = bc(csb_sb[HALF:, tsl])
                        nc.vector.tensor_mul(t1[:], x1, cosb)    # x1*cos
                        nc.vector.tensor_mul(t4[:], x1, sinb)    # x1*sin
                        nc.vector.tensor_mul(t2[:], x2, sinb2)   # x2*sin
                        nc.vector.tensor_mul(t3[:], x2, cosb2)   # x2*cos
                        nc.vector.tensor_sub(x1, t1[:], t2[:])
                        nc.vector.tensor_add(x2, t3[:], t4[:])

                    def rope_k(dst):
                        # in-place neox rope on dst[0:64, t] (2D)
                        x1 = dst[:HALF]
                        x2 = dst[HALF:RD]
                        sh = [HALF, QC]
                        t1 = ropep.tile(sh, BF16, name="k1", tag="k1", bufs=1)
                        t2 = ropep.tile(sh, BF16, name="k2", tag="k2", bufs=1)
                        t3 = ropep.tile(sh, BF16, name="k3", tag="k3", bufs=1)
                        t4 = ropep.tile(sh, BF16, name="k4", tag="k4", bufs=1)
                        nc.vector.tensor_mul(t1[:], x1, csa_sb[:HALF, tsl])
                        nc.vector.tensor_mul(t4[:], x1, csb_sb[:HALF, tsl])
                        nc.vector.tensor_mul(t2[:], x2, csa_sb[HALF:, tsl])
                        nc.vector.tensor_mul(t3[:], x2, csb_sb[HALF:, tsl])
                        nc.vector.tensor_sub(x1, t1[:], t2[:])
                        nc.vector.tensor_add(x2, t3[:], t4[:])

                    # one j-tile per PSUM-buf round so the 2-buf rotation
                    # double-buffers evictions (ht is SBUF-resident; re-reads free)
                    for j in range(NJ):
                        ps = psq.tile([128, QC], F32, name=f"pq{j}", tag="qkv", bufs=2)
                        for ko in range(NKO):
                            nc.tensor.matmul(
                                ps[:],
                                w_sb[:, ko, j * HD:(j + 1) * HD],
                                ht_tiles[(tci, ko >> 3)][:, ko & 7, :],
                                start=(ko == 0),
                                stop=(ko == NKO - 1),
                            )
                        if j < HPC:
                            nc.scalar.copy(qcur[:, j, :], ps[:])
                            if j % 2 == 1:
                                rope_pair(qcur[:, j - 1:j + 1, :])
                        elif j == HPC:
                            nc.scalar.copy(kT_tiles[tci][:], ps[:])
                            rope_k(kT_tiles[tci][:])
                        else:
                            vt = miscp.tile([128, QC], BF16, name="vt", tag="vt", bufs=2)
                            nc.scalar.copy(vt[:], ps[:])
                            nc.sync.dma_start_transpose(v_tiles[tci][:], vt[:])

                def do_attn(tci):
                    """causal attention for q-chunk tci, 4 heads in 2-head passes."""
                    nkt = 4 * tci + 4
                    for pp in range(2):  # head pair
                        av = psa.tile([128, 2, QC], F32, name="av", tag="av", bufs=1)
                        acc = exp_pool.tile([128, 2, QC], BF16, name="acc", tag="acc", bufs=2)
                        for kt in range(nkt):
                            _o = kt - 4 * tci
                            qoff = 0 if _o < 0 else _o * 128
                            qs = slice(qoff, QC)
                            ss = psa.tile([128, 2, QC], F32, name="ss", tag="ss", bufs=2)
                            for i in range(2):
                                nc.tensor.matmul(
                                    ss[:, i, qs],
                                    kT_tiles[kt >> 2][:, (kt & 3) * 128:((kt & 3) + 1) * 128],
                                    qcur_tiles[tci][:, 2 * pp + i, qs],
                                    start=True,
                                    stop=True,
                                )
                            ex = exp_pool.tile([128, 2, QC], BF16, name="ex", tag="ex", bufs=3)
                            nc.scalar.activation(ex[:, :, qs], ss[:, :, qs], EXP)
                            if _o >= 0:
                                # triangular mask on the 128 diagonal columns
                                nc.vector.scalar_tensor_tensor(
                                    ex[:, :, qoff:qoff + 128],
                                    ex[:, :, qoff:qoff + 128],
                                    1.0,
                                    dmask_sb[:],
                                    op0=MUL,
                                    op1=MUL,
                                )
                            if kt == 0:
                                nc.vector.tensor_copy(acc[:], ex[:])
                            else:
                                nc.vector.tensor_add(
                                    acc[:, :, qs], ex[:, :, qs], acc[:, :, qs]
                                )
                            for i in range(2):
                                nc.tensor.matmul(
                                    av[:, i, qs],
                                    v_tiles[kt >> 2][:, kt & 3, :],
                                    ex[:, i, qs],
                                    start=(kt == 0),
                                    stop=(kt == nkt - 1),
                                )
                        # denominator: partition-reduce acc, broadcast to 128 rows
                        dn = psa.tile([128, 2, QC], F32, name="dn", tag="ss", bufs=2)
                        for i in range(2):
                            nc.tensor.matmul(
                                dn[:, i, :], ones_sb[:], acc[:, i, :],
                                start=True, stop=True,
                            )
                        # evict av raw (fast ACT copy frees the 2 PSUM banks for the
                        # next pass ~2.5us earlier than normalizing from PSUM would)
                        avr = exp_pool.tile([128, 2, QC], BF16, name="avr", tag="avr", bufs=2)
                        nc.scalar.copy(avr[:], av[:])
                        # 1/dn: single-op DVE approx (~18 bits; full reciprocal()
                        # is a ~6.5us multi-pass uop program)
                        rd = miscp.tile([128, 2, QC], F32, name="rd", tag="rd", bufs=2)
                        nc.vector.reciprocal_approx_fast(rd[:], dn[:])
                        nc.vector.scalar_tensor_tensor(
                            attn_tiles[tci][:, 2 * pp:2 * pp + 2, :],
                            avr[:], 1.0, rd[:], op0=MUL, op1=MUL,
                        )

                for tci in range(NTC):
                    do_qkv(tci)
                    # remaining quarters of tci+1, then head of tci+2
                    load_ht_q(tci + 1, 2)
                    load_ht_q(tci + 1, 3)
                    load_ht_q(tci + 2, 0)
                    load_ht_q(tci + 2, 1)
                    do_attn(tci)

            # ---- o_proj partial (out_p = attn_part.T @ w_op), fp32 psum, bf16 out
            with tc.tile_pool(name="wo", bufs=1) as wop, \
                 tc.tile_pool(name="p3", bufs=1) as p3p, \
                 tc.tile_pool(name="pso", bufs=1, space="PSUM") as pso:
                wo_sb = wop.tile([128, HPC, HIDDEN], BF16, name="wo")
                wo_view = w_op[:].rearrange("(h d) o -> d h o", d=128)
                for oc in range(8):
                    osl = slice(oc * QC, (oc + 1) * QC)
                    nc.sync.dma_start(wo_sb[:, :, osl], wo_view[:, :, osl])
                for oc in range(8):
                    osl = slice(oc * QC, (oc + 1) * QC)
                    for tl in range(T // 128):
                        po = pso.tile([128, QC], F32, name="po", tag="po", bufs=2)
                        for h in range(HPC):
                            nc.tensor.matmul(
                                po[:],
                                attn_tiles[tl >> 2][:, h, (tl & 3) * 128:((tl & 3) + 1) * 128],
                                wo_sb[:, h, osl],
                                start=(h == 0),
                                stop=(h == HPC - 1),
                            )
                        ob = p3p.tile([128, QC], BF16, name="ob", tag="ob", bufs=6)
                        if tl % 2 == 0:
                            nc.vector.tensor_copy(ob[:], po[:])
                        else:
                            nc.scalar.copy(ob[:], po[:])
                        nc.sync.dma_start(
                            out_p[tl * 128:(tl + 1) * 128, osl], ob[:]
                        )
    nc.compile()
    return nc


def _host_prep(positions, hidden_states, w_qkv, w_o):
    import ml_dtypes

    BF = ml_dtypes.bfloat16
    positions = np.asarray(positions)
    hidden_states = np.asarray(hidden_states, dtype=np.float32)
    w_qkv = np.asarray(w_qkv, dtype=np.float32)
    w_o = np.asarray(w_o, dtype=np.float32)

    hidden_t = np.ascontiguousarray(hidden_states.T).astype(BF)

    pos = positions.astype(np.float32)
    r = np.arange(0, RD, 2, dtype=np.float32) / np.float32(RD)
    inv_freq = (np.float32(1.0) / (np.float32(ROPE_BASE) ** r)).astype(np.float32)
    ang = pos[:, None] * inv_freq[None, :]
    cos_t = np.cos(ang).astype(np.float32).T  # [32, T]
    sin_t = np.sin(ang).astype(np.float32).T
    # A = [cos;sin], B = [sin;cos]  (base-partition-aligned rope products)
    cs_a = np.ascontiguousarray(np.concatenate([cos_t, sin_t], axis=0)).astype(BF)
    cs_b = np.ascontiguousarray(np.concatenate([sin_t, cos_t], axis=0)).astype(BF)

    p = np.arange(128, dtype=np.int64)[:, None]
    x = np.arange(128, dtype=np.int64)[None, :]
    tri = (x >= p).astype(np.float32)
    dmask2 = np.ascontiguousarray(np.tile(tri, (1, 2))).astype(BF)

    scale = np.float32(HD ** -0.5)
    q_size = NH * HD
    kv_size = NKV * HD
    in_maps = []
    for c in range(NC_CORES):
        wq = w_qkv[:, c * HPC * HD:(c + 1) * HPC * HD] * scale
        wk = w_qkv[:, q_size + c * HD:q_size + (c + 1) * HD]
        wv = w_qkv[:, q_size + kv_size + c * HD:q_size + kv_size + (c + 1) * HD]
        w_qkvp = np.ascontiguousarray(
            np.concatenate([wq, wk, wv], axis=1)
        ).astype(BF)
        w_op = np.ascontiguousarray(w_o[c * HPC * HD:(c + 1) * HPC * HD, :]).astype(BF)
        in_maps.append(
            {
                "hidden_t": hidden_t,
                "w_qkvp": w_qkvp,
                "w_op": w_op,
                "cs_a": cs_a,
                "cs_b": cs_b,
                "dmask2": dmask2,
            }
        )
    return in_maps


def kernel(positions, hidden_states, w_qkv, w_o, _trace=False, _trace_kw=None):
    from concourse.bass_utils import run_bass_kernel_spmd

    if "nc" not in _CACHE:
        _CACHE["nc"] = _build_nc()
    nc = _CACHE["nc"]

    in_maps = _host_prep(positions, hidden_states, w_qkv, w_o)
    kw = dict(_trace_kw or {})
    res = run_bass_kernel_spmd(
        nc, in_maps, list(range(NC_CORES)), trace=_trace, **kw
    )
    out = np.zeros((T, HIDDEN), np.float32)
    for c in range(NC_CORES):
        out += res.results[c]["out_p"].astype(np.float32)
    if _trace:
        _CACHE["last_exec_time_ns"] = res.exec_time_ns
        _CACHE["last_results"] = res
    return out


# revision 24
# speedup vs baseline: 1.0137x; 1.0137x over previous
"""Trainium2 Bass kernel for MiniMax softmax attention (T=4096, H=4096, 32 q heads,
8 kv heads, head_dim=128, partial neox RoPE, causal softmax, o_proj).

Sharding: tensor-parallel over heads across 8 NeuronCores. Core c computes q heads
4c..4c+3 (= kv-head group c). Host sums the 8 partial outputs (row-parallel o_proj).

Design (measured 850us vs 1232us f32r baseline; PE ~810us busy of 778us pure
matmul streaming -> ~95% tensor-engine utilization):
  * all matmuls in bf16 (fp32 PSUM accumulation). f32r matmuls self-load weights
    serially (~287ns/MM measured vs 213ns stream floor); bf16 pipelines LDWEIGHTS
    under the previous MM stream (~227ns/MM measured). Accuracy gate is 2e-2;
    this lands at ~8e-3.
  * softmax denominator: no 128x1xN PE matmuls per key-tile (148us of PE in the
    baseline). exp tiles are accumulated lane-wise on DVE in bf16 via tensor_add
    (2x_1p mode, ~623ns per 2-head key-tile), then ONE ones[128,128] matmul per
    (head-pair, q-chunk) partition-reduces the accumulator and broadcasts the
    denominator to 128 partitions; 1/dn via reciprocal_approx_fast (the full
    reciprocal() is a ~6.5us multi-pass uop program; ln/exp on ACT thrashes
    activation-table loads).
  * o_proj fully fused on-chip: normalized attention stays in SBUF (bf16),
    no DRAM spill round-trip; output partials written bf16 (halves writeback).
  * v transposed via DMA xbar transpose (off-engine) instead of PE transposes.
  * attention runs in 2-head passes (av 2 + ss 2x2 = 6 PSUM banks) leaving 2
    banks so the next chunk's qkv projection overlaps the ACT-bound softmax.
  * hidden_t chunks are cached in SBUF (bf16, 6 quarter-buffers, prefetched 1.5
    chunks ahead) so qkv runs j-tile-at-a-time on 2 rotating PSUM banks with
    no HBM re-reads and no eviction stalls.
"""
import numpy as np

T = 4096
HIDDEN = 4096
NH = 32
NKV = 8
HD = 128
RD = 64
HALF = 32
ROPE_BASE = 10000000.0
NC_CORES = 8
HPC = NH // NC_CORES      # 4 q heads per core
QC = 512                  # query chunk
NTC = T // QC             # 8 t-chunks
NKO = 32                  # hidden contraction chunks of 128
NJ = HPC + 2              # 6 j-tiles of 128 per core

_CACHE = {}


def _build_nc():
    import concourse.mybir as mybir
    import concourse.tile as tile
    from concourse import bacc

    F32 = mybir.dt.float32
    BF16 = mybir.dt.bfloat16
    EXP = mybir.ActivationFunctionType.Exp
    LOG = mybir.ActivationFunctionType.Ln
    MUL = mybir.AluOpType.mult
    ADD = mybir.AluOpType.add

    from concourse.bass import broadcast_tensor_aps

    nc = bacc.Bacc()
    hidden_t = nc.dram_tensor("hidden_t", [HIDDEN, T], BF16, kind="ExternalInput")
    w_qkvp = nc.dram_tensor("w_qkvp", [HIDDEN, NJ * HD], BF16, kind="ExternalInput")
    w_op = nc.dram_tensor("w_op", [HPC * HD, HIDDEN], BF16, kind="ExternalInput")
    cs_a = nc.dram_tensor("cs_a", [RD, T], BF16, kind="ExternalInput")
    cs_b = nc.dram_tensor("cs_b", [RD, T], BF16, kind="ExternalInput")
    dmask2 = nc.dram_tensor("dmask2", [128, 2 * 128], BF16, kind="ExternalInput")
    out_p = nc.dram_tensor("out_p", [T, HIDDEN], BF16, kind="ExternalOutput")

    with tile.TileContext(nc) as tc:
        with (
            tc.tile_pool(name="const", bufs=1) as const,
            tc.tile_pool(name="kv", bufs=1) as kvp,
            tc.tile_pool(name="ht", bufs=1) as htp,
            tc.tile_pool(name="qt", bufs=1) as qtp,
            tc.tile_pool(name="rope", bufs=1) as ropep,
            tc.tile_pool(name="ex", bufs=1) as exp_pool,
            tc.tile_pool(name="misc", bufs=1) as miscp,
            tc.tile_pool(name="attn", bufs=1) as attnp,
            tc.tile_pool(name="psa", bufs=1, space="PSUM") as psa,
        ):
            # ---- constants (DMAs for cs/dmask deferred below the first ht/w
            # loads so the first qkv matmul isn't queued behind them)
            csa_sb = const.tile([RD, T], BF16, name="csa", tag="csa")
            csb_sb = const.tile([RD, T], BF16, name="csb", tag="csb")
            dmask_sb = const.tile([128, 2, 128], BF16, name="dmask", tag="dmask")
            ones_sb = const.tile([128, 128], BF16, name="ones", tag="ones")
            nc.gpsimd.memset(ones_sb[:], 1.0)

            # kv cache: one tile per t-chunk to keep dependency tracking fine-grained
            kT_tiles = []     # [128 d, 512 keys] bf16 (roped)
            v_tiles = []      # [128 keys(sub), 4 sub, 128 d] bf16
            attn_tiles = []   # [128 d, 4 h, 512 t] bf16 normalized attention^T
            for i in range(NTC):
                kT_tiles.append(kvp.tile([128, QC], BF16, name=f"kT{i}", tag=f"kT{i}"))
                v_tiles.append(kvp.tile([128, 4, HD], BF16, name=f"v{i}", tag=f"v{i}"))
                attn_tiles.append(
                    attnp.tile([128, HPC, QC], BF16, name=f"at{i}", tag=f"at{i}")
                )

            with tc.tile_pool(name="w", bufs=1) as wp, \
                 tc.tile_pool(name="psq", bufs=1, space="PSUM") as psq:
                w_sb = wp.tile([128, NKO, NJ * HD], BF16, name="w")
                w_view = w_qkvp[:].rearrange("(ko p) j -> p ko j", p=128)

                ht_view = hidden_t[:].rearrange("(ko p) t -> p ko t", p=128)
                ht_tiles = {}

                def load_ht_q(tci, q):
                    if tci >= NTC or (tci, q) in ht_tiles:
                        return
                    tsl = slice(tci * QC, (tci + 1) * QC)
                    htt = htp.tile([128, 8, QC], BF16, name="ht", tag="ht", bufs=6)
                    nc.sync.dma_start(htt[:], ht_view[:, q * 8:(q + 1) * 8, tsl])
                    ht_tiles[(tci, q)] = htt

                # first compute deps first: ht(0) quarter 0 + w chunk 0
                nc.sync.dma_start(w_sb[:, 0:4, :], w_view[:, 0:4, :])
                for q in range(4):
                    load_ht_q(0, q)
                for wi in range(1, 8):
                    nc.sync.dma_start(
                        w_sb[:, wi * 4:(wi + 1) * 4, :], w_view[:, wi * 4:(wi + 1) * 4, :]
                    )
                nc.sync.dma_start(csa_sb[:], cs_a[:])
                nc.sync.dma_start(csb_sb[:], cs_b[:])
                nc.sync.dma_start(
                    dmask_sb[:], dmask2[:].rearrange("p (two t) -> p two t", two=2)
                )
                load_ht_q(1, 0)
                load_ht_q(1, 1)
                qcur_tiles = {}

                def do_qkv(tci):
                    """qkv^T projection for t-chunk tci + rope + kv-cache fill."""
                    tsl = slice(tci * QC, (tci + 1) * QC)
                    qcur = qtp.tile([128, HPC, QC], BF16, name="qcur", tag="qt", bufs=2)
                    qcur_tiles[tci] = qcur

                    def rope_pair(dst):
                        # in-place neox rope on dst[0:64, pair, t]; dst bf16
                        # dst: [128, 2, 512]; tables broadcast over the pair dim
                        x1 = dst[:HALF]
                        x2 = dst[HALF:RD]
                        sh = [HALF, 2, QC]
                        t1 = ropep.tile(sh, BF16, name="r1", tag="r1", bufs=1)
                        t2 = ropep.tile(sh, BF16, name="r2", tag="r2", bufs=1)
                        t3 = ropep.tile(sh, BF16, name="r3", tag="r3", bufs=1)
                        t4 = ropep.tile(sh, BF16, name="r4", tag="r4", bufs=1)

                        def bc(tab):  # [32, 512] -> [32, 2(0-stride), 512]
                            t3d = tab.rearrange("p (one t) -> p one t", one=1)
                            b, _ = broadcast_tensor_aps(t3d, x1)
                            return b

                        cosb = bc(csa_sb[:HALF, tsl])
                        sinb2 = bc(csa_sb[HALF:, tsl])
                        sinb = bc(csb_sb[:HALF, tsl])
                        cosb2 = bc(csb_sb[HALF:, tsl])
                        nc.vector.tensor_mul(t1[:], x1, cosb)    # x1*cos
                        nc.vector.tensor_mul(t4[:], x1, sinb)    # x1*sin
                        nc.vector.tensor_mul(t2[:], x2, sinb2)   # x2*sin
                        nc.vector.tensor_mul(t3[:], x2, cosb2)   # x2*cos
                        nc.vector.tensor_sub(x1, t1[:], t2[:])
                        nc.vector.tensor_add(x2, t3[:], t4[:])

                    def rope_k(dst):
                        # in-place neox rope on dst[0:64, t] (2D)
                        x1 = dst[:HALF]
                        x2 = dst[HALF:RD]
                        sh = [HALF, QC]
                        t1 = ropep.tile(sh, BF16, name="k1", tag="k1", bufs=1)
                        t2 = ropep.tile(sh, BF16, name="k2", tag="k2", bufs=1)
                        t3 = ropep.tile(sh, BF16, name="k3", tag="k3", bufs=1)
                        t4 = ropep.tile(sh, BF16, name="k4", tag="k4", bufs=1)
                        nc.vector.tensor_mul(t1[:], x1, csa_sb[:HALF, tsl])
                        nc.vector.tensor_mul(t4[:], x1, csb_sb[:HALF, tsl])
                        nc.vector.tensor_mul(t2[:], x2, csa_sb[HALF:, tsl])
                        nc.vector.tensor_mul(t3[:], x2, csb_sb[HALF:, tsl])
                        nc.vector.tensor_sub(x1, t1[:], t2[:])
                        nc.vector.tensor_add(x2, t3[:], t4[:])

                    # one j-tile per PSUM-buf round so the 2-buf rotation
                    # double-buffers evictions (ht is SBUF-resident; re-reads free)
                    for j in range(NJ):
                        ps = psq.tile([128, QC], F32, name=f"pq{j}", tag="qkv", bufs=2)
                        for ko in range(NKO):
                            nc.tensor.matmul(
                                ps[:],
                                w_sb[:, ko, j * HD:(j + 1) * HD],
                                ht_tiles[(tci, ko >> 3)][:, ko & 7, :],
                                start=(ko == 0),
                                stop=(ko == NKO - 1),
                            )
                        if j < HPC:
                            nc.scalar.copy(qcur[:, j, :], ps[:])
                            if j % 2 == 1:
                                rope_pair(qcur[:, j - 1:j + 1, :])
                        elif j == HPC:
                            nc.scalar.copy(kT_tiles[tci][:], ps[:])
                            rope_k(kT_tiles[tci][:])
                        else:
                            vt = miscp.tile([128, QC], BF16, name="vt", tag="vt", bufs=2)
                            nc.scalar.copy(vt[:], ps[:])
                            nc.sync.dma_start_transpose(v_tiles[tci][:], vt[:])

                def do_attn(tci):
                    """causal attention for q-chunk tci, 4 heads in 2-head passes."""
                    nkt = 4 * tci + 4
                    for pp in range(2):  # head pair
                        av = psa.tile([128, 2, QC], F32, name="av", tag="av", bufs=1)
                        acc = exp_pool.tile([128, 2, QC], BF16, name="acc", tag="acc", bufs=2)
                        for kt in range(nkt):
                            _o = kt - 4 * tci
                            qoff = 0 if _o < 0 else _o * 128
                            qs = slice(qoff, QC)
                            ss = psa.tile([128, 2, QC], F32, name="ss", tag="ss", bufs=2)
                            for i in range(2):
                                nc.tensor.matmul(
                                    ss[:, i, qs],
                                    kT_tiles[kt >> 2][:, (kt & 3) * 128:((kt & 3) + 1) * 128],
                                    qcur_tiles[tci][:, 2 * pp + i, qs],
                                    start=True,
                                    stop=True,
                                )
                            ex = exp_pool.tile([128, 2, QC], BF16, name="ex", tag="ex", bufs=3)
                            nc.scalar.activation(ex[:, :, qs], ss[:, :, qs], EXP)
                            if _o >= 0:
                                # triangular mask on the 128 diagonal columns
                                nc.vector.scalar_tensor_tensor(
                                    ex[:, :, qoff:qoff + 128],
                                    ex[:, :, qoff:qoff + 128],
                                    1.0,
                                    dmask_sb[:],
                                    op0=MUL,
                                    op1=MUL,
                                )
                            if kt == 0:
                                nc.vector.tensor_copy(acc[:], ex[:])
                            else:
                                nc.vector.tensor_add(
                                    acc[:, :, qs], ex[:, :, qs], acc[:, :, qs]
                                )
                            for i in range(2):
                                nc.tensor.matmul(
                                    av[:, i, qs],
                                    v_tiles[kt >> 2][:, kt & 3, :],
                                    ex[:, i, qs],
                                    start=(kt == 0),
                                    stop=(kt == nkt - 1),
                                )
                        # denominator: partition-reduce acc, broadcast to 128 rows
                        dn = psa.tile([128, 2, QC], F32, name="dn", tag="ss", bufs=2)
                        for i in range(2):
                            nc.tensor.matmul(
                                dn[:, i, :], ones_sb[:], acc[:, i, :],
                                start=True, stop=True,
                            )
                        # 1/dn: single-op DVE approx (~18 bits; full reciprocal()
                        # is a ~6.5us multi-pass uop program)
                        rd = miscp.tile([128, 2, QC], F32, name="rd", tag="rd", bufs=2)
                        nc.vector.reciprocal_approx_fast(rd[:], dn[:])
                        nc.vector.scalar_tensor_tensor(
                            attn_tiles[tci][:, 2 * pp:2 * pp + 2, :],
                            av[:], 1.0, rd[:], op0=MUL, op1=MUL,
                        )

                for tci in range(NTC):
                    do_qkv(tci)
                    # remaining quarters of tci+1, then head of tci+2
                    load_ht_q(tci + 1, 2)
                    load_ht_q(tci + 1, 3)
                    load_ht_q(tci + 2, 0)
                    load_ht_q(tci + 2, 1)
                    do_attn(tci)

            # ---- o_proj partial (out_p = attn_part.T @ w_op), fp32 psum, bf16 out
            with tc.tile_pool(name="wo", bufs=1) as wop, \
                 tc.tile_pool(name="p3", bufs=1) as p3p, \
                 tc.tile_pool(name="pso", bufs=1, space="PSUM") as pso:
                wo_sb = wop.tile([128, HPC, HIDDEN], BF16, name="wo")
                wo_view = w_op[:].rearrange("(h d) o -> d h o", d=128)
                for oc in range(8):
                    osl = slice(oc * QC, (oc + 1) * QC)
                    nc.sync.dma_start(wo_sb[:, :, osl], wo_view[:, :, osl])
                for oc in range(8):
                    osl = slice(oc * QC, (oc + 1) * QC)
                    for tl in range(T // 128):
                        po = pso.tile([128, QC], F32, name="po", tag="po", bufs=2)
                        for h in range(HPC):
                            nc.tensor.matmul(
                                po[:],
                                attn_tiles[tl >> 2][:, h, (tl & 3) * 128:((tl & 3) + 1) * 128],
                                wo_sb[:, h, osl],
                                start=(h == 0),
                                stop=(h == HPC - 1),
                            )
                        ob = p3p.tile([128, QC], BF16, name="ob", tag="ob", bufs=6)
                        if tl % 2 == 0:
                            nc.vector.tensor_copy(ob[:], po[:])
                        else:
                            nc.scalar.copy(ob[:], po[:])
                        nc.sync.dma_start(
                            out_p[tl * 128:(tl + 1) * 128, osl], ob[:]
                        )
    nc.compile()
    return nc


def _host_prep(positions, hidden_states, w_qkv, w_o):
    import ml_dtypes

    BF = ml_dtypes.bfloat16
    positions = np.asarray(positions)
    hidden_states = np.asarray(hidden_states, dtype=np.float32)
    w_qkv = np.asarray(w_qkv, dtype=np.float32)
    w_o = np.asarray(w_o, dtype=np.float32)

    hidden_t = np.ascontiguousarray(hidden_states.T).astype(BF)

    pos = positions.astype(np.float32)
    r = np.arange(0, RD, 2, dtype=np.float32) / np.float32(RD)
    inv_freq = (np.float32(1.0) / (np.float32(ROPE_BASE) ** r)).astype(np.float32)
    ang = pos[:, None] * inv_freq[None, :]
    cos_t = np.cos(ang).astype(np.float32).T  # [32, T]
    sin_t = np.sin(ang).astype(np.float32).T
    # A = [cos;sin], B = [sin;cos]  (base-partition-aligned rope products)
    cs_a = np.ascontiguousarray(np.concatenate([cos_t, sin_t], axis=0)).astype(BF)
    cs_b = np.ascontiguousarray(np.concatenate([sin_t, cos_t], axis=0)).astype(BF)

    p = np.arange(128, dtype=np.int64)[:, None]
    x = np.arange(128, dtype=np.int64)[None, :]
    tri = (x >= p).astype(np.float32)
    dmask2 = np.ascontiguousarray(np.tile(tri, (1, 2))).astype(BF)

    scale = np.float32(HD ** -0.5)
    q_size = NH * HD
    kv_size = NKV * HD
    in_maps = []
    for c in range(NC_CORES):
        wq = w_qkv[:, c * HPC * HD:(c + 1) * HPC * HD] * scale
        wk = w_qkv[:, q_size + c * HD:q_size + (c + 1) * HD]
        wv = w_qkv[:, q_size + kv_size + c * HD:q_size + kv_size + (c + 1) * HD]
        w_qkvp = np.ascontiguousarray(
            np.concatenate([wq, wk, wv], axis=1)
        ).astype(BF)
        w_op = np.ascontiguousarray(w_o[c * HPC * HD:(c + 1) * HPC * HD, :]).astype(BF)
        in_maps.append(
            {
                "hidden_t": hidden_t,
                "w_qkvp": w_qkvp,
                "w_op": w_op,
                "cs_a": cs_a,
                "cs_b": cs_b,
                "dmask2": dmask2,
            }
        )
    return in_maps


def kernel(positions, hidden_states, w_qkv, w_o, _trace=False, _trace_kw=None):
    from concourse.bass_utils import run_bass_kernel_spmd

    if "nc" not in _CACHE:
        _CACHE["nc"] = _build_nc()
    nc = _CACHE["nc"]

    in_maps = _host_prep(positions, hidden_states, w_qkv, w_o)
    kw = dict(_trace_kw or {})
    res = run_bass_kernel_spmd(
        nc, in_maps, list(range(NC_CORES)), trace=_trace, **kw
    )
    out = np.zeros((T, HIDDEN), np.float32)
    for c in range(NC_CORES):
        out += res.results[c]["out_p"].astype(np.float32)
    if _trace:
        _CACHE["last_exec_time_ns"] = res.exec_time_ns
        _CACHE["last_results"] = res
    return out
